# revision 4
# baseline (speedup 1.0000x reference)
"""Trainium2 Bass kernel for nn_ConvFCLIFNet.

Pipeline: x_seq (T=64, B=512, 1, 28, 28) -> conv2x2(valid) -> FC(729) -> LIF
scan over T -> spike sequence (T, B, 729) in {0.0, 1.0}.

Strategy (v2)
-------------
- conv + FC + bias + 1/tau fold into ONE matmul: y*0.5 = x_aug @ W_aug where
  x_aug = [x_pixels(784), 1.0] and W_aug[p, o] = 0.5 * (fc_w @ C)^T (C = conv
  scatter), bias row at p=784.
- Data-parallel over 8 NeuronCores: 64 samples each.
- Contraction (785 rows) split as 6x112 + 113 so every k-tile rounds up to
  the PE's 128-row tile mode: no (32,128) mode switches, which reset the PE
  p-state ramp and pin the array at 1.2 GHz (measured: uniform full-row tiles
  sustain ~227 ns per 512-col matmul vs 427 ns with a 17-row tail tile).
- T=64 processed in quanta of [8,16,16,16,8] timesteps. Per quantum and
  feature chunk j, matmuls accumulate into a PSUM slot, which Scalar/GpSimd
  immediately copy to an SBUF y-buffer. The LIF scan runs entirely from SBUF,
  so the tensor engine's matmul stream is never gated on the scan.
- LIF scan: ONE custom DVE op per timestep:
      u = (q_prev == SENT) ? 0 : q_prev;  w = z + u
      q = (w >= 1) ? SENT : 0.5 * w
  Spike decode on GpSimd: s = (q >= 1.0) -> exactly 1.0 iff spiked.
- Outputs batched 4 timesteps per DMA into a partition-major DRAM layout
  [128, T, NJ, BS] (contiguous 6 KiB per partition per DMA).
"""
import numpy as np

import concourse.bacc as bacc
import concourse.mybir as mybir
import concourse.tile as tile
from concourse.bass_utils import run_bass_kernel_spmd

# ---------------------------------------------------------------- constants
T, B, H, W = 64, 512, 28, 28
NPIX = H * W            # 784
NROWS = NPIX + 1        # 785 contraction rows (pixels + bias)
NF = 729                # fc features
NCORES = 8
BS = B // NCORES        # 64 samples per core
NJ = 6                  # feature chunks of 128 (768 padded)
KT = 7                  # contraction k-tiles: 6 x 112 + 113
KROW = 112              # rows per k-tile (kt < 6)
KTAIL = NROWS - 6 * KROW   # 113 (112 pixels + bias row)
QUANTA = (8, 16, 16, 16, 8)   # timesteps per quantum
OUTB = 4                # timesteps per output DMA
SENT = float(2 ** 20)

_CACHE = {}

# ------------------------------------------------------------ custom DVE op

def _register_lif_op():
    from concourse.dve_spec import Spec, Src0, Src1, C0, C1, Zero, One, select, eq, lower
    from concourse.dve_uop import DveOpSpec
    from concourse import dve_ops

    name = "LIF_STEP_ANT"
    for op in dve_ops.OPS:
        if op.name == name:
            return op

    def _ref(in0, in1, s0, s1, imm2=None):
        u = np.where(in1 == s0, 0.0, in1).astype(np.float32)
        w = (in0 + u).astype(np.float32)
        return np.where(w >= 1.0, np.float32(s0), (w * np.float32(s1)).astype(np.float32))

    _u = select(eq(Src1, C0), Zero, Src1)
    _w = Src0 + _u
    spec = Spec(body=select(_w >= One, C0, _w * C1), reference=_ref)

    row = dve_ops._CUSTOM_DVE_ROW_BASE + len(dve_ops.OPS)
    assert row < 0x20
    dve_ops._SUB_OPCODE_FOR_NAME[name] = row
    shas = {}
    for ver in ("v3", "v4"):
        s = DveOpSpec(name=name, opcode=row, uops=lower(spec, ver=ver), rd1_en=True)
        shas[ver] = s.sha(ver)
    op = dve_ops.DveOp(name, spec, subdim=False, uops_sha=shas)
    dve_ops.OPS.append(op)
    dve_ops.CUSTOM_DVE_SPECS[name] = spec
    return op

# ------------------------------------------------------------- device build

def _build():
    lif = _register_lif_op()
    nc = bacc.Bacc(None, target_bir_lowering=False, debug=False)
    f32, f32r = mybir.dt.float32, mybir.dt.float32r
    NTOT = T * BS  # 4096 moving columns total
    with tile.TileContext(nc) as tc:
        with tc.tile_pool(name="dram", bufs=1, space="DRAM") as dram, \
             tc.tile_pool(name="consts", bufs=1) as consts, \
             tc.tile_pool(name="xpool", bufs=2) as xpool, \
             tc.tile_pool(name="ypool", bufs=2) as ypool, \
             tc.tile_pool(name="qpool", bufs=2) as qpool, \
             tc.tile_pool(name="spool", bufs=3) as spool, \
             tc.tile_pool(name="pspool", bufs=4, space="PSUM") as pspool:
            x_in = dram.tile([NROWS, NTOT], f32r, kind="ExternalInput",
                             name="x_in", uniquify=False)
            w_in = dram.tile([NJ, 128, KT, 128], f32r, kind="ExternalInput",
                             name="w_in", uniquify=False)
            out = dram.tile([128, T, NJ, BS], f32, kind="ExternalOutput",
                            name="out", uniquify=False)

            # weights: per-j DMA so j=0 lands first and matmuls start early
            wsb = consts.tile([128, NJ, KT, 128], f32r)
            for j in range(NJ):
                nc.sync.dma_start(out=wsb[:, j, :, :], in_=w_in[j])

            q = qpool.tile([128, NJ, BS], f32, name="q", tag="q")
            nc.vector.memset(q[:, :, :], 0.0)

            # prefetch x for quantum 0
            t0s = np.cumsum((0,) + QUANTA)

            def load_x(c):
                tq = QUANTA[c]
                c0, c1 = t0s[c] * BS, t0s[c + 1] * BS
                x_sb = xpool.tile([128, KT, 16 * BS], f32r, name="x_sb", tag="x")
                nc.sync.dma_start(
                    out=x_sb[0:KROW, 0:6, 0:(c1 - c0)],
                    in_=x_in[0:6 * KROW, c0:c1].rearrange(
                        "(k p) n -> p k n", p=KROW),
                )
                nc.sync.dma_start(
                    out=x_sb[0:KTAIL, 6, 0:(c1 - c0)],
                    in_=x_in[6 * KROW:NROWS, c0:c1],
                )
                return x_sb

            x_tiles = {0: load_x(0)}

            for c, tq in enumerate(QUANTA):
                ns = tq * BS                      # moving cols this quantum
                nh = ns // 512                    # psum-bank halves
                x_sb = x_tiles.pop(c)
                if c + 1 < len(QUANTA):
                    x_tiles[c + 1] = load_x(c + 1)
                y_sb = ypool.tile([128, NJ, 16 * BS], f32, name="y_sb", tag="y")
                for j in range(NJ):
                    ps = pspool.tile([128, 1024], f32, name="ps", tag="ps")
                    for h in range(nh):
                        sl = slice(h * 512, (h + 1) * 512)
                        for kt in range(6):
                            nc.tensor.matmul(
                                ps[:, sl],
                                lhsT=wsb[0:KROW, j, kt, :],
                                rhs=x_sb[0:KROW, kt, sl],
                                start=(kt == 0), stop=False,
                            )
                        nc.tensor.matmul(
                            ps[:, sl],
                            lhsT=wsb[0:KTAIL, j, 6, :],
                            rhs=x_sb[0:KTAIL, 6, sl],
                            start=False, stop=True,
                        )
                    # drain PSUM slot to SBUF right away (GpSimd can't read
                    # PSUM on TRN2, so Scalar does all copies; GpSimd decodes)
                    for h in range(nh):
                        sl = slice(h * 512, (h + 1) * 512)
                        nc.scalar.copy(out=y_sb[:, j, sl], in_=ps[:, sl])

                # LIF scan over this quantum's timesteps (from SBUF)
                for tl in range(tq):
                    t = t0s[c] + tl
                    q2 = qpool.tile([128, NJ, BS], f32, name="q", tag="q")
                    nc.vector._custom_dve(
                        lif,
                        out=q2[:, :, :],
                        in0=y_sb[:, :, tl * BS:(tl + 1) * BS],
                        in1=q[:, :, :],
                        s0=SENT, s1=0.5,
                    )
                    if t % OUTB == 0:
                        s4 = spool.tile([128, OUTB, NJ, BS], f32,
                                        name="s4", tag="s")
                    nc.gpsimd.tensor_scalar(
                        out=s4[:, t % OUTB, :, :], in0=q2[:, :, :],
                        scalar1=1.0, scalar2=None, op0=mybir.AluOpType.is_ge,
                    )
                    if t % OUTB == OUTB - 1:
                        nc.sync.dma_start(
                            out=out[:, t - OUTB + 1:t + 1, :, :],
                            in_=s4[:, :, :, :])
                    q = q2
    nc.compile()
    return nc

# --------------------------------------------------------------- host side

def _prep_weights(conv_w, fc_w, fc_b):
    """w_in [NJ, 128, KT, 128]: contraction rows (785 = 6*112 + 113) in
    partition-major per-j blocks; cols = 768 features (729 + pad); scaled by
    0.5 (tau fold). Bias lives at global row 784 (kt=6, p=112)."""
    cw = conv_w.reshape(2, 2).astype(np.float32)
    fcw = fc_w.astype(np.float32).reshape(NF, 27, 27)
    tmp = np.zeros((NF, H, W), np.float32)
    for dr in range(2):
        for dc in range(2):
            tmp[:, dr:dr + 27, dc:dc + 27] += cw[dr, dc] * fcw
    w_eff = tmp.reshape(NF, NPIX)                     # [729, 784]
    w_aug = np.zeros((KT * 128, NJ * 128), np.float32)
    # rows: kt*128 + p  <->  global contraction row kt*112 + p (p < rows_kt)
    for kt in range(KT):
        rows = KROW if kt < 6 else KTAIL
        g0 = kt * KROW
        for p in range(rows):
            g = g0 + p
            if g < NPIX:
                w_aug[kt * 128 + p, :NF] = 0.5 * w_eff[:, g]
            elif g == NPIX:
                w_aug[kt * 128 + p, :NF] = 0.5 * fc_b.astype(np.float32)
    w4 = w_aug.reshape(KT, 128, NJ, 128)
    return np.ascontiguousarray(w4.transpose(2, 1, 0, 3))  # [NJ, 128, KT, 128]

def _prep_x(x_seq):
    """Per-core pixel-major inputs [NCORES][785, T*BS], cols t-major."""
    xs = x_seq.reshape(T, NCORES, BS, NPIX)
    xt = xs.transpose(1, 3, 0, 2).reshape(NCORES, NPIX, T * BS)
    xp = np.empty((NCORES, NROWS, T * BS), np.float32)
    xp[:, :NPIX, :] = xt
    xp[:, NPIX, :] = 1.0
    return xp

def kernel(x_seq, conv_w, fc_w, fc_b):
    if "nc" not in _CACHE:
        _CACHE["nc"] = _build()
    nc = _CACHE["nc"]
    w_in = _prep_weights(conv_w, fc_w, fc_b)
    xp = _prep_x(np.asarray(x_seq, dtype=np.float32))
    in_maps = [{"x_in": np.ascontiguousarray(xp[c]), "w_in": w_in}
               for c in range(NCORES)]
    res = run_bass_kernel_spmd(nc, in_maps, core_ids=list(range(NCORES)))
    _CACHE["last_res"] = res
    full = np.empty((T, B, NF), np.float32)
    for c in range(NCORES):
        o = res.results[c]["out"]                     # [128, T, NJ, BS]
        # feature f = j*128 + p ; sample s
        full[:, c * BS:(c + 1) * BS, :] = (
            o.transpose(1, 3, 2, 0).reshape(T, BS, NJ * 128)[:, :, :NF])
    return full


# revision 9
# speedup vs baseline: 2.4334x; 2.4334x over previous
"""Trainium2 Bass kernel for nn_ConvFCLIFNet.

Pipeline: x_seq (T=64, B=512, 1, 28, 28) -> conv2x2(valid) -> FC(729) -> LIF
scan over T -> spike sequence (T, B, 729) in {0.0, 1.0}.

Strategy (v2)
-------------
- conv + FC + bias + 1/tau fold into ONE matmul: y*0.5 = x_aug @ W_aug where
  x_aug = [x_pixels(784), 1.0] and W_aug[p, o] = 0.5 * (fc_w @ C)^T (C = conv
  scatter), bias row at p=784.
- Data-parallel over 8 NeuronCores: 64 samples each.
- Contraction (785 rows) split as 6x112 + 113 so every k-tile rounds up to
  the PE's 128-row tile mode: no (32,128) mode switches, which reset the PE
  p-state ramp and pin the array at 1.2 GHz (measured: uniform full-row tiles
  sustain ~227 ns per 512-col matmul vs 427 ns with a 17-row tail tile).
- T=64 processed in quanta of [8,16,16,16,8] timesteps. Per quantum and
  feature chunk j, matmuls accumulate into a PSUM slot, which Scalar/GpSimd
  immediately copy to an SBUF y-buffer. The LIF scan runs entirely from SBUF,
  so the tensor engine's matmul stream is never gated on the scan.
- LIF scan: ONE custom DVE op per timestep:
      u = (q_prev == SENT) ? 0 : q_prev;  w = z + u
      q = (w >= 1) ? SENT : 0.5 * w
  No on-device spike decode: the raw sentinel-encoded state q is DMA'd out
  (batched 4 timesteps per DMA, partition-major DRAM layout [128, T, NJ, BS])
  and the host decodes spike = (q == SENT). 0.5*w < 0.5 always, so the
  sentinel is unambiguous.
"""
import numpy as np

import concourse.bacc as bacc
import concourse.mybir as mybir
import concourse.tile as tile
from concourse.bass_utils import run_bass_kernel_spmd

# ---------------------------------------------------------------- constants
T, B, H, W = 64, 512, 28, 28
NPIX = H * W            # 784
NROWS = NPIX + 1        # 785 contraction rows (pixels + bias)
NF = 729                # fc features
NCORES = 8
BS = B // NCORES        # 64 samples per core
NJ = 6                  # feature chunks of 128 (768 padded)
KT = 7                  # contraction k-tiles: 6 x 112 + 113
KROW = 112              # rows per k-tile (kt < 6)
KTAIL = NROWS - 6 * KROW   # 113 (112 pixels + bias row)
QUANTA = (8, 16, 16, 16, 8)   # timesteps per quantum
OUTB = 4                # timesteps per output DMA
SENT = float(2 ** 20)

_CACHE = {}

# ------------------------------------------------------------ custom DVE op

def _register_lif_op():
    from concourse.dve_spec import Spec, Src0, Src1, C0, C1, Zero, One, select, eq, lower
    from concourse.dve_uop import DveOpSpec
    from concourse import dve_ops

    name = "LIF_STEP_ANT"
    for op in dve_ops.OPS:
        if op.name == name:
            return op

    def _ref(in0, in1, s0, s1, imm2=None):
        u = np.where(in1 == s0, 0.0, in1).astype(np.float32)
        w = (in0 + u).astype(np.float32)
        return np.where(w >= 1.0, np.float32(s0), (w * np.float32(s1)).astype(np.float32))

    _u = select(eq(Src1, C0), Zero, Src1)
    _w = Src0 + _u
    spec = Spec(body=select(_w >= One, C0, _w * C1), reference=_ref)

    row = dve_ops._CUSTOM_DVE_ROW_BASE + len(dve_ops.OPS)
    assert row < 0x20
    dve_ops._SUB_OPCODE_FOR_NAME[name] = row
    shas = {}
    for ver in ("v3", "v4"):
        s = DveOpSpec(name=name, opcode=row, uops=lower(spec, ver=ver), rd1_en=True)
        shas[ver] = s.sha(ver)
    op = dve_ops.DveOp(name, spec, subdim=False, uops_sha=shas)
    dve_ops.OPS.append(op)
    dve_ops.CUSTOM_DVE_SPECS[name] = spec
    return op

# ------------------------------------------------------------- device build

def _build():
    lif = _register_lif_op()
    nc = bacc.Bacc(None, target_bir_lowering=False, debug=False)
    f32, f32r = mybir.dt.float32, mybir.dt.float32r
    NTOT = T * BS  # 4096 moving columns total
    with tile.TileContext(nc) as tc:
        with tc.tile_pool(name="dram", bufs=1, space="DRAM") as dram, \
             tc.tile_pool(name="consts", bufs=1) as consts, \
             tc.tile_pool(name="xpool", bufs=2) as xpool, \
             tc.tile_pool(name="ypool", bufs=2) as ypool, \
             tc.tile_pool(name="pspool", bufs=4, space="PSUM") as pspool:
            x_in = dram.tile([NROWS, NTOT], f32r, kind="ExternalInput",
                             name="x_in", uniquify=False)
            w_in = dram.tile([NJ, 128, KT, 128], f32r, kind="ExternalInput",
                             name="w_in", uniquify=False)
            out = dram.tile([128, T, NJ, BS], f32, kind="ExternalOutput",
                            name="out", uniquify=False)

            # weights: per-j DMA so j=0 lands first and matmuls start early
            wsb = consts.tile([128, NJ, KT, 128], f32r)
            for j in range(NJ):
                nc.sync.dma_start(out=wsb[:, j, :, :], in_=w_in[j])

            # 8-slot ring of LIF state; slot t%8 holds q after step t. DMA'd
            # out 4 slots at a time. Slot 7 doubles as the zero initial state.
            qring = consts.tile([128, 8, NJ, BS], f32)
            nc.vector.memset(qring[:, 7, :, :], 0.0)

            # prefetch x for quantum 0
            t0s = np.cumsum((0,) + QUANTA)

            def load_x(c):
                tq = QUANTA[c]
                c0, c1 = t0s[c] * BS, t0s[c + 1] * BS
                x_sb = xpool.tile([128, KT, 16 * BS], f32r, name="x_sb", tag="x")
                nc.sync.dma_start(
                    out=x_sb[0:KROW, 0:6, 0:(c1 - c0)],
                    in_=x_in[0:6 * KROW, c0:c1].rearrange(
                        "(k p) n -> p k n", p=KROW),
                )
                nc.sync.dma_start(
                    out=x_sb[0:KTAIL, 6, 0:(c1 - c0)],
                    in_=x_in[6 * KROW:NROWS, c0:c1],
                )
                return x_sb

            x_tiles = {0: load_x(0)}

            for c, tq in enumerate(QUANTA):
                ns = tq * BS                      # moving cols this quantum
                nh = ns // 512                    # psum-bank halves
                x_sb = x_tiles.pop(c)
                if c + 1 < len(QUANTA):
                    x_tiles[c + 1] = load_x(c + 1)
                y_sb = ypool.tile([128, NJ, 16 * BS], f32, name="y_sb", tag="y")
                for j in range(NJ):
                    ps = pspool.tile([128, 1024], f32, name="ps", tag="ps")
                    for h in range(nh):
                        sl = slice(h * 512, (h + 1) * 512)
                        for kt in range(6):
                            nc.tensor.matmul(
                                ps[:, sl],
                                lhsT=wsb[0:KROW, j, kt, :],
                                rhs=x_sb[0:KROW, kt, sl],
                                start=(kt == 0), stop=False,
                            )
                        nc.tensor.matmul(
                            ps[:, sl],
                            lhsT=wsb[0:KTAIL, j, 6, :],
                            rhs=x_sb[0:KTAIL, 6, sl],
                            start=False, stop=True,
                        )
                    # drain PSUM slot to SBUF right away (GpSimd can't read
                    # PSUM on TRN2, so Scalar does all copies; GpSimd decodes)
                    for h in range(nh):
                        sl = slice(h * 512, (h + 1) * 512)
                        nc.scalar.copy(out=y_sb[:, j, sl], in_=ps[:, sl])

                # LIF scan over this quantum's timesteps (from SBUF)
                for tl in range(tq):
                    t = t0s[c] + tl
                    nc.vector._custom_dve(
                        lif,
                        out=qring[:, t % 8, :, :],
                        in0=y_sb[:, :, tl * BS:(tl + 1) * BS],
                        in1=qring[:, (t - 1) % 8, :, :],
                        s0=SENT, s1=0.5,
                    )
                    if t % OUTB == OUTB - 1:
                        b = (t - OUTB + 1) % 8
                        nc.sync.dma_start(
                            out=out[:, t - OUTB + 1:t + 1, :, :],
                            in_=qring[:, b:b + OUTB, :, :])
    nc.compile()
    return nc

# --------------------------------------------------------------- host side

def _prep_weights(conv_w, fc_w, fc_b):
    """w_in [NJ, 128, KT, 128]: contraction rows (785 = 6*112 + 113) in
    partition-major per-j blocks; cols = 768 features (729 + pad); scaled by
    0.5 (tau fold). Bias lives at global row 784 (kt=6, p=112)."""
    cw = conv_w.reshape(2, 2).astype(np.float32)
    fcw = fc_w.astype(np.float32).reshape(NF, 27, 27)
    tmp = np.zeros((NF, H, W), np.float32)
    for dr in range(2):
        for dc in range(2):
            tmp[:, dr:dr + 27, dc:dc + 27] += cw[dr, dc] * fcw
    w_eff = tmp.reshape(NF, NPIX)                     # [729, 784]
    w_aug = np.zeros((KT * 128, NJ * 128), np.float32)
    # rows: kt*128 + p  <->  global contraction row kt*112 + p (p < rows_kt)
    for kt in range(KT):
        rows = KROW if kt < 6 else KTAIL
        g0 = kt * KROW
        for p in range(rows):
            g = g0 + p
            if g < NPIX:
                w_aug[kt * 128 + p, :NF] = 0.5 * w_eff[:, g]
            elif g == NPIX:
                w_aug[kt * 128 + p, :NF] = 0.5 * fc_b.astype(np.float32)
    w4 = w_aug.reshape(KT, 128, NJ, 128)
    return np.ascontiguousarray(w4.transpose(2, 1, 0, 3))  # [NJ, 128, KT, 128]

def _prep_x(x_seq):
    """Per-core pixel-major inputs [NCORES][785, T*BS], cols t-major."""
    xs = x_seq.reshape(T, NCORES, BS, NPIX)
    xt = xs.transpose(1, 3, 0, 2).reshape(NCORES, NPIX, T * BS)
    xp = np.empty((NCORES, NROWS, T * BS), np.float32)
    xp[:, :NPIX, :] = xt
    xp[:, NPIX, :] = 1.0
    return xp

def kernel(x_seq, conv_w, fc_w, fc_b):
    if "nc" not in _CACHE:
        _CACHE["nc"] = _build()
    nc = _CACHE["nc"]
    w_in = _prep_weights(conv_w, fc_w, fc_b)
    xp = _prep_x(np.asarray(x_seq, dtype=np.float32))
    in_maps = [{"x_in": np.ascontiguousarray(xp[c]), "w_in": w_in}
               for c in range(NCORES)]
    res = run_bass_kernel_spmd(nc, in_maps, core_ids=list(range(NCORES)))
    _CACHE["last_res"] = res
    full = np.empty((T, B, NF), np.float32)
    for c in range(NCORES):
        o = res.results[c]["out"]                     # [128, T, NJ, BS]
        # spike decode: q == SENT exactly iff the neuron fired this step
        s = (o == np.float32(SENT)).astype(np.float32)
        # feature f = j*128 + p ; sample s
        full[:, c * BS:(c + 1) * BS, :] = (
            s.transpose(1, 3, 2, 0).reshape(T, BS, NJ * 128)[:, :, :NF])
    return full
